# revision 2
# baseline (speedup 1.0000x reference)
"""Trainium2 Bass kernel for nn_MultiHeadAttentionLayer (GNN message passing).

Strategy (edge-parallel, dst-sorted, all-streaming):
  - Host: sort edges by dst; pack them into "supertiles" of 1024 edges
    (128 partitions x 8 subtiles) that are node-aligned and whose dst values
    span a <= 63-node window; assign contiguous supertile ranges to the 8
    cores (balanced edge counts, disjoint dst ranges -> no collectives).
  - Sharded inputs per core (replicated node features per the edge-parallel
    scheme, transposed fp16 tiles): h[src].T, h[dst].T, e.T per subtile with a
    ones-row for bias augmentation, plus a one-hot dst-slot selector.
  - Device per supertile: TensorE projects K|V, Q, pe per-edge (stationary =
    data tiles, moving = augmented weights); VectorE computes the score chain;
    ScalarE the exp; GpSimd the clamp; TensorE segment-sums s*V and s into a
    PSUM node-window via selector matmuls. A small pass 2 divides wV by
    (z + 1e-6).
  - Host: unpermute e_out rows and scatter h_out window rows (the unshard).
"""
import numpy as np

N = 50000
E = 1600000
P = 128
JPT = 8
EDGES_PER_ST = 1024
WMAX = 63
N_CORES = 8
H, D = 8, 8

_kernel_cache = {}


# ---------------------------------------------------------------- host plan --
def _build_plan(src, dst):
    perm = np.argsort(dst, kind="stable")
    dst_s = dst[perm].astype(np.int64)

    n_nodes = int(dst.max()) + 1 if len(dst) else 0
    n_nodes = max(n_nodes, N)
    deg = np.bincount(dst_s, minlength=n_nodes)
    node_start = np.concatenate([[0], np.cumsum(deg)])
    nwe = np.nonzero(deg)[0]
    assert deg.max() <= EDGES_PER_ST, "node degree beyond one supertile unsupported"

    supertiles = []
    i, M = 0, len(nwe)
    while i < M:
        base = nwe[i]
        e0 = node_start[base]
        jn = i
        while jn < M and nwe[jn] - base + 1 <= WMAX and node_start[nwe[jn] + 1] - e0 <= EDGES_PER_ST:
            jn += 1
        last = nwe[jn - 1]
        supertiles.append((int(e0), int(node_start[last + 1]), int(base), int(last - base + 1)))
        i = jn

    n_st = len(supertiles)
    st_edges = np.array([s[1] - s[0] for s in supertiles])
    cum = np.cumsum(st_edges)
    bounds = [0]
    for c in range(1, N_CORES):
        bounds.append(int(np.searchsorted(cum, cum[-1] * c / N_CORES)))
    bounds.append(n_st)
    S = max(bounds[c + 1] - bounds[c] for c in range(N_CORES))
    S = ((S + 1) // 2) * 2  # even, for pass-2 row grouping

    cores = []
    for c in range(N_CORES):
        sts = supertiles[bounds[c]:bounds[c + 1]]
        n_real = len(sts)
        eids = np.full((S, EDGES_PER_ST), -1, np.int64)
        slots = np.full((S, EDGES_PER_ST), WMAX, np.int64)
        st_base = np.zeros(S, np.int64)
        st_span = np.zeros(S, np.int64)
        sorted_pos = np.full((S, EDGES_PER_ST), -1, np.int64)
        for t, (e0, e1, base, span) in enumerate(sts):
            ne = e1 - e0
            eids[t, :ne] = perm[e0:e1]
            slots[t, :ne] = dst_s[e0:e1] - base
            sorted_pos[t, :ne] = np.arange(e0, e1)
            st_base[t], st_span[t] = base, span
        cores.append(dict(eids=eids, slots=slots, st_base=st_base, st_span=st_span,
                          sorted_pos=sorted_pos, n_real=n_real))
    return dict(perm=perm, src=src.astype(np.int64), dst=dst.astype(np.int64),
                S=S, cores=cores)


def _make_streams(plan, h, e, c):
    """fp16 device streams for core c: hsT/hdT/esT [S, 65, 1024], ssel [S, 128, 512]."""
    core = plan["cores"][c]
    S = plan["S"]
    eids = core["eids"]
    safe = np.maximum(eids, 0)
    src, dst = plan["src"], plan["dst"]

    def to_T(x):
        # x [S, 1024, 64] f32; edge k=(p, j): p=k//8, j=k%8
        x4 = x.reshape(S, P, JPT, 64)
        out = np.empty((S, 65, JPT, P), np.float16)
        out[:, :64] = x4.transpose(0, 3, 2, 1)
        out[:, 64] = 1.0
        return out.reshape(S, 65, JPT * P)

    hsT = to_T(h[src[safe]])
    hdT = to_T(h[dst[safe]])
    esT = to_T(e[safe])

    slots = core["slots"].reshape(S, P, JPT)
    ssel = np.zeros((S, P, JPT, 64), np.float16)
    ss, pp, jj = np.ogrid[:S, :P, :JPT]
    ssel[ss, pp, jj, slots] = 1.0
    return hsT, hdT, esT, ssel.reshape(S, P, 512)


def _make_weights(Qw, Qb, Kw, Kb, Vw, Vb, Ew, Eb):
    scale = np.float32(1.0 / np.sqrt(D))
    wkv = np.zeros((65, 128), np.float32)
    wkv[:64, :64] = Kw.T
    wkv[64, :64] = Kb
    wkv[:64, 64:] = Vw.T
    wkv[64, 64:] = Vb
    wq = np.vstack([Qw.T, Qb[None]])
    we = np.vstack([Ew.T, Eb[None]]) * scale
    return wkv.astype(np.float16), wq.astype(np.float16), we.astype(np.float16)


# ------------------------------------------------------------- device build --
def _build_kernel(S, repeat=1):
    import concourse.bass as bass
    import concourse.bacc as bacc
    import concourse.mybir as mybir
    from concourse.tile import TileContext

    F16 = mybir.dt.float16
    F32 = mybir.dt.float32
    AX = mybir.AxisListType
    OP = mybir.AluOpType
    AF = mybir.ActivationFunctionType

    R = S * 64
    G_TOT = R // 128
    CH2 = 10
    n2 = (G_TOT + CH2 - 1) // CH2

    nc = bacc.Bacc("TRN2", target_bir_lowering=False, debug=False, num_devices=N_CORES)
    hsT = nc.declare_dram_parameter("hsT", [S, 65, 1024], F16, isOutput=False)
    hdT = nc.declare_dram_parameter("hdT", [S, 65, 1024], F16, isOutput=False)
    esT = nc.declare_dram_parameter("esT", [S, 65, 1024], F16, isOutput=False)
    ssel = nc.declare_dram_parameter("ssel", [S, 128, 512], F16, isOutput=False)
    wkv = nc.declare_dram_parameter("wkv", [65, 128], F16, isOutput=False)
    wq = nc.declare_dram_parameter("wq", [65, 64], F16, isOutput=False)
    we = nc.declare_dram_parameter("we", [65, 64], F16, isOutput=False)
    eout = nc.declare_dram_parameter("eout", [S, 128, 512], F16, isOutput=True)
    hout = nc.declare_dram_parameter("hout", [R, 64], F32, isOutput=True)
    wvz = nc.dram_tensor("wvz", [S, 64, 72], F32)

    with TileContext(nc) as tc:
        with (
            tc.tile_pool(name="consts", bufs=1) as cp,
            tc.tile_pool(name="streams", bufs=3) as sp,
            tc.tile_pool(name="work", bufs=3) as wp,
            tc.tile_pool(name="pskv", bufs=2, space="PSUM") as pskv,
            tc.tile_pool(name="psqp", bufs=1, space="PSUM") as psqp,
            tc.tile_pool(name="pswz", bufs=2, space="PSUM") as pswz,
        ):
            w_kv = cp.tile([65, 128], F16)
            w_q = cp.tile([65, 64], F16)
            w_e = cp.tile([65, 64], F16)
            nc.sync.dma_start(out=w_kv[:], in_=wkv[:])
            nc.sync.dma_start(out=w_q[:], in_=wq[:])
            nc.sync.dma_start(out=w_e[:], in_=we[:])

            for _rep in range(repeat):
                for t in range(S):
                    t_hs = sp.tile([65, 1024], F16, tag="hs")
                    t_hd = sp.tile([65, 1024], F16, tag="hd")
                    t_es = sp.tile([65, 1024], F16, tag="es")
                    t_ss = sp.tile([128, 512], F16, tag="ss")
                    nc.sync.dma_start(out=t_hs[:], in_=hsT[t])
                    nc.sync.dma_start(out=t_hd[:], in_=hdT[t])
                    nc.scalar.dma_start(out=t_es[:], in_=esT[t])
                    nc.scalar.dma_start(out=t_ss[:], in_=ssel[t])

                    ps_kv = pskv.tile([128, 8, 128], F32, tag="kv")
                    ps_qd = psqp.tile([128, 8, 64], F32, tag="qd")
                    ps_pe = psqp.tile([128, 8, 64], F32, tag="pe")
                    for j in range(8):
                        nc.tensor.matmul(out=ps_kv[:, j, :], lhsT=t_hs[:, bass.ts(j, 128)],
                                         rhs=w_kv[:], start=True, stop=True)
                        nc.tensor.matmul(out=ps_qd[:, j, :], lhsT=t_hd[:, bass.ts(j, 128)],
                                         rhs=w_q[:], start=True, stop=True)
                        nc.tensor.matmul(out=ps_pe[:, j, :], lhsT=t_es[:, bass.ts(j, 128)],
                                         rhs=w_e[:], start=True, stop=True)

                    qd_sb = wp.tile([128, 8, 64], F16, tag="qdsb")
                    nc.scalar.activation(out=qd_sb[:], in_=ps_qd[:], func=AF.Copy)
                    score1 = wp.tile([128, 8, 64], F16, tag="sc1")
                    nc.vector.tensor_tensor(out=score1[:], in0=ps_kv[:, :, 0:64],
                                            in1=qd_sb[:], op=OP.mult)
                    score = wp.tile([128, 8, 64], F16, tag="sc")
                    nc.vector.tensor_tensor(out=score[:], in0=score1[:],
                                            in1=ps_pe[:], op=OP.mult)
                    nc.sync.dma_start(out=eout[t], in_=score.rearrange("p j f -> p (j f)"))

                    ssum = wp.tile([128, 64], F32, tag="ssum")
                    nc.vector.tensor_reduce(out=ssum[:],
                                            in_=score.rearrange("p j (h d) -> p (j h) d", d=8),
                                            axis=AX.X, op=OP.add)
                    sclip = wp.tile([128, 64], F32, tag="sclip")
                    nc.gpsimd.tensor_scalar(out=sclip[:], in0=ssum[:], scalar1=5.0,
                                            scalar2=-5.0, op0=OP.min, op1=OP.max)
                    sv = wp.tile([128, 8, 72], F16, tag="sv")
                    nc.scalar.activation(out=sv[:, :, 64:72],
                                         in_=sclip.rearrange("p (j h) -> p j h", j=8),
                                         func=AF.Exp)
                    nc.vector.tensor_tensor(
                        out=sv[:, :, 0:64].rearrange("p j (h d) -> p j h d", d=8),
                        in0=ps_kv[:, :, 64:128].rearrange("p j (h d) -> p j h d", d=8),
                        in1=sv[:, :, 64:72].to_broadcast([128, 8, 8, 8]),
                        op=OP.mult)

                    ps_wz = pswz.tile([64, 72], F32, tag="wz")
                    for j in range(8):
                        nc.tensor.matmul(out=ps_wz[:], lhsT=t_ss[:, bass.ts(j, 64)],
                                         rhs=sv[:, j, :], start=(j == 0), stop=(j == 7))
                    wz_sb = wp.tile([64, 72], F32, tag="wzsb")
                    nc.scalar.activation(out=wz_sb[:], in_=ps_wz[:], func=AF.Copy)
                    nc.scalar.dma_start(out=wvz[t], in_=wz_sb[:])

            wvz_rows = wvz.rearrange("s n f -> (s n) f").rearrange("(g p) f -> g p f", p=128)
            hout_rows = hout.rearrange("(g p) f -> g p f", p=128)
            for it in range(n2):
                g0 = it * CH2
                g1 = min(g0 + CH2, G_TOT)
                gg = g1 - g0
                wz = wp.tile([128, gg, 72], F32, tag="p2wz")
                nc.sync.dma_start(out=wz[:], in_=wvz_rows[g0:g1].rearrange("g p f -> p g f"))
                zr = wp.tile([128, gg, 8], F32, tag="p2zr")
                nc.vector.tensor_scalar(out=zr[:], in0=wz[:, :, 64:72], scalar1=1e-6,
                                        scalar2=None, op0=OP.add)
                nc.vector.reciprocal(out=zr[:], in_=zr[:])
                ho = wp.tile([128, gg, 64], F32, tag="p2ho")
                nc.vector.tensor_tensor(out=ho.rearrange("p g (h d) -> p g h d", d=8),
                                        in0=wz[:, :, 0:64].rearrange("p g (h d) -> p g h d", d=8),
                                        in1=zr.to_broadcast([128, gg, 8, 8]),
                                        op=OP.mult)
                nc.sync.dma_start(out=hout_rows[g0:g1].rearrange("g p f -> p g f"), in_=ho[:])
    nc.compile()
    return nc


# ------------------------------------------------------------------- driver --
def kernel(h, e, src, dst, Qw, Qb, Kw, Kb, Vw, Vb, Ew, Eb):
    from concourse.bass_utils import run_bass_kernel_spmd

    h = np.asarray(h, np.float32)
    e = np.asarray(e, np.float32)
    src = np.asarray(src, np.int32)
    dst = np.asarray(dst, np.int32)

    plan = _build_plan(src, dst)
    S = plan["S"]
    wkv, wq, we = _make_weights(np.asarray(Qw, np.float32), np.asarray(Qb, np.float32),
                                np.asarray(Kw, np.float32), np.asarray(Kb, np.float32),
                                np.asarray(Vw, np.float32), np.asarray(Vb, np.float32),
                                np.asarray(Ew, np.float32), np.asarray(Eb, np.float32))

    in_maps = []
    for c in range(N_CORES):
        hsT, hdT, esT, sselv = _make_streams(plan, h, e, c)
        in_maps.append(dict(hsT=hsT, hdT=hdT, esT=esT, ssel=sselv,
                            wkv=wkv, wq=wq, we=we))

    if S not in _kernel_cache:
        _kernel_cache[S] = _build_kernel(S)
    nc = _kernel_cache[S]

    res = run_bass_kernel_spmd(nc, in_maps, list(range(N_CORES)))

    e_out = np.zeros((E, 64), np.float32)
    h_out = np.zeros((N, 64), np.float32)
    for c in range(N_CORES):
        core = plan["cores"][c]
        eo = np.asarray(res.results[c]["eout"], np.float32).reshape(S, P, JPT, 64)
        ho = np.asarray(res.results[c]["hout"]).reshape(S, 64, 64)
        pos = core["sorted_pos"].reshape(S, P, JPT)
        valid = pos >= 0
        e_out[plan["perm"][pos[valid]]] = eo[valid]
        for t in range(core["n_real"]):
            b, sp_ = core["st_base"][t], core["st_span"][t]
            h_out[b:b + sp_] = ho[t, :sp_]
    return h_out.reshape(N, H, D), e_out.reshape(E, H, D)
